# revision 36
# baseline (speedup 1.0000x reference)
"""Trainium2 Bass kernel for the 2D Gaussian splatting model (nn_GaussianModel2D).

Math (per pixel p, gaussians n = 0..255 in order):
    e_n(p)   = -(a dx^2 + 2b dx dy + c dy^2) + ln(opac_n)      (quadratic in x,y)
    alpha_n  = exp(e_n)            (clip to 0.99 never binds for this input;
                                    checked on host, fallback applies it)
    u_n      = 1 - alpha_n
    scan_t   = prod_{k<=t} u_k = T_{t+1}  (inclusive cumprod)
    out_c    = clip(c0_c + sum_t gamma_{t,c} * scan_t, 0, 1)
    where gamma_{t,c} = col_{t+1,c} - col_{t,c} (t<255), gamma_{255,c} = 1 - col_{255,c}
    (Abel summation of sum_n w_n col_n + bg;  w_n = T_n - T_{n+1}, bg = T_256)

Device layout per core (1/8 of the image rows = 32768 pixels):
    - exponent: PE matmul, K=6 features [x'^2, x'y', y'^2, x', y', 1] (centered),
      fp32r, out (128 pix, 256 g) chunks in PSUM
    - alpha: ACT exp, PSUM->SBUF fp16, 4 chunks per call
    - u = 1 - alpha: DVE tensor_scalar fp16 (fast 2-byte mode)
    - scan_t: the DVE scan runs at ~2.1 ns/element, so the gaussian axis is
      permuted host-side to [evens | odds] per chunk and the cumprod is done
      over SHIFTED PAIR PRODUCTS, halving scan elements:
        w_0     = u_0,  w_k = u_{2k-1} * u_{2k}   (DVE tensor_tensor, 2B fast)
        S_{2k}  = cumprod(w)_k                    (segmented scan, see below)
        S_{2k+1}= S_{2k} * u_{2k+1}               (DVE tensor_tensor mult)
      (u_{2k-1} is read as a shifted in-tile view; the k=0 edge is patched by
      an 8-element copy per super-batch.  A divide-based variant would avoid
      the shift but AluOpType.divide is not a valid DVE TensorTensor ISA op.)
      The scan uses a segmented-reset formulation, ONE instruction per 8
      chunks: state = max(mask_t, state_prev) * w_t, where mask = 1.0 at each
      chunk's first slot and 0.0 elsewhere.  Since all intermediate products
      lie in (0, 1], max(1, state) = 1 restarts the cumprod exactly at
      128-column boundaries, so 8 independent per-pixel cumprods run in a
      single long instruction (per-instruction scan overhead dominated the
      old per-chunk version).
    - transpose scan (pix, g) -> (g, pix): ONE batched DMA xbar transpose per
      8 chunks ([128, 2048] -> [128, 16, 128] per-128-block transposes); the
      per-DMA issue cost on the SP sequencer (~1.2 us) dominated the old
      per-half version (512 issues -> 32).
    - rendered: PE matmul gamma^T @ scan^T accumulated in PSUM at partition
      offsets 32m (zero-padded gamma to 32 cols so PSUM is fully written),
      rhs strided over 4 chunks per instruction (512 moving columns)
    - final: DVE tensor_scalar (+c0, min 1) PSUM->SBUF staging [128, 2048],
      then one DMA per 32-partition block per 64 chunks (16 output DMAs)
"""

import numpy as np

H, W, N = 512, 512, 256
NCORES = 8
ROWS_PER_CORE = H // NCORES            # 64
PIX = ROWS_PER_CORE * W                # 32768 pixels per core
CHUNK = 128                            # pixels per matmul chunk
NCHUNK = PIX // CHUNK                  # 256
SB = 16                                # chunks per super-batch (scan+transpose)
NSB = NCHUNK // SB                     # 16
SCAN_F = SB * N // 2                   # 2048 (scan runs over pair products)

_CACHE = {}


def _build_program(apply_opacity_clip: bool):
    import concourse.bass as bass
    import concourse.bacc as bacc
    import concourse.tile as tile
    import concourse.mybir as mybir
    from contextlib import ExitStack

    fp32 = mybir.dt.float32
    fp32r = mybir.dt.float32r
    fp16 = mybir.dt.float16
    Alu = mybir.AluOpType
    Act = mybir.ActivationFunctionType

    nc = bacc.Bacc("TRN2", target_bir_lowering=False, debug=False,
                   num_devices=NCORES)

    ft_d = nc.dram_tensor("ft", [6, PIX], fp32r, kind="ExternalInput")
    c6_d = nc.dram_tensor("c6", [6, N], fp32r, kind="ExternalInput")
    gam_d = nc.dram_tensor("gam", [N, 32], fp16, kind="ExternalInput")
    c0_d = nc.dram_tensor("c0", [128, 1], fp32, kind="ExternalInput")
    msk_d = nc.dram_tensor("msk", [128, SCAN_F], fp16, kind="ExternalInput")
    out_d = nc.dram_tensor("out", [3, PIX], fp32, kind="ExternalOutput")

    with tile.TileContext(nc) as tc, ExitStack() as ctx:
        consts = ctx.enter_context(tc.tile_pool(name="consts", bufs=1))
        ftpool = ctx.enter_context(tc.tile_pool(name="ftp", bufs=3))
        apool = ctx.enter_context(tc.tile_pool(name="alpha", bufs=3))
        upool = ctx.enter_context(tc.tile_pool(name="u", bufs=3))
        vpool = ctx.enter_context(tc.tile_pool(name="v", bufs=3))
        tpool = ctx.enter_context(tc.tile_pool(name="tsc", bufs=3))
        ttpool = ctx.enter_context(tc.tile_pool(name="tt", bufs=2))
        opool = ctx.enter_context(tc.tile_pool(name="osb", bufs=2))
        eps_pool = ctx.enter_context(tc.tile_pool(name="eps", bufs=3, space="PSUM"))
        rps_pool = ctx.enter_context(tc.tile_pool(name="rps", bufs=2, space="PSUM"))

        c6_sb = consts.tile([6, N], fp32r)
        gam_sb = consts.tile([128, 2, 32], fp16)
        c0_sb = consts.tile([128, 1], fp32)
        msk_sb = consts.tile([128, SCAN_F], fp16)
        nc.sync.dma_start(c6_sb[:], c6_d[:])
        nc.sync.dma_start(gam_sb[:], gam_d.ap().rearrange("(b k) c -> k b c", k=128))
        nc.sync.dma_start(c0_sb[:], c0_d[:])
        nc.sync.dma_start(msk_sb[:], msk_d[:])

        QCH = NCHUNK // 4                          # chunks per ft quarter
        # Quarter 0 on the fast HWDGE queue (no transposes in flight yet, so
        # no ring conflict); quarters 1-3 issued upfront on the gpsimd SWDGE
        # queue, which keeps these large loads off the HWDGE ring the xbar
        # transposes use (concurrent big descriptors there corrupt in-flight
        # transposes).  The pool WAR semaphores naturally delay Q2/Q3 until
        # their buffer frees.
        ft_tiles = []

        def load_quarter(Q):
            # two half-loads with 8KB descriptors: the first super-batch's
            # matmuls unblock on the first half, and small descriptors don't
            # monopolize DMA engines against the in-flight xbar transposes
            ft_sb = ftpool.tile([6, QCH * CHUNK], fp32r, name="ftq")
            src = ft_d.ap()[:, Q * QCH * CHUNK:(Q + 1) * QCH * CHUNK]
            half = QCH * CHUNK // 2
            nc.gpsimd.dma_start(ft_sb[:, :half], src[:, :half],
                                max_dma_last_dim=2048)
            nc.gpsimd.dma_start(ft_sb[:, half:], src[:, half:],
                                max_dma_last_dim=2048)
            ft_tiles.append(ft_sb)

        def stage_a(sb):
            ft_sb = ft_tiles[(sb * SB) // QCH]
            u2 = upool.tile([128, SB * N], fp16, name="u2")
            for h in range(SB // 4):
                e_ps = eps_pool.tile([128, 4 * N], fp32, name="e_ps")
                for q in range(4):
                    j = (sb * SB + h * 4 + q) % QCH
                    nc.tensor.matmul(
                        e_ps[:, q * N:(q + 1) * N],
                        lhsT=ft_sb[:, j * CHUNK:(j + 1) * CHUNK],
                        rhs=c6_sb[:],
                        start=True, stop=True)
                al = apool.tile([128, 4 * N], fp16, name="al")
                nc.scalar.activation(al[:], e_ps[:], Act.Exp)
                if apply_opacity_clip:
                    nc.vector.tensor_scalar(al[:], al[:], 0.99, None, Alu.min)
                nc.vector.tensor_scalar(
                    u2[:, h * 4 * N:(h + 1) * 4 * N], al[:], -1.0, 1.0,
                    Alu.mult, Alu.add)
            return u2

        def stage_b(sb, u2):
            # per-chunk gaussian layout (after host permutation of C6 cols):
            # u2[:, c*256 : c*256+128] = u_even, u2[:, c*256+128 : +256] = u_odd
            u3 = u2.rearrange("p (c b g) -> p b c g", b=2, g=CHUNK)
            uf = u2.rearrange("p (c g) -> p c g", g=2 * CHUNK)
            w = vpool.tile([128, SCAN_F], fp16, name="w")
            w3 = w.rearrange("p (c g) -> p c g", g=CHUNK)
            t_sc = tpool.tile([128, 2 * SCAN_F], fp16, name="t_sc")
            # SB0 runs its w-prep + scan in two 8-chunk halves so the DVE can
            # start as soon as the first half of u2 lands (startup latency);
            # the segmented-reset mask makes the half-scans exact
            nparts = 2 if sb == 0 else 1
            cpp = SB // nparts
            for hh in range(nparts):
                cs = slice(hh * cpp, (hh + 1) * cpp)
                fs = slice(hh * cpp * CHUNK, (hh + 1) * cpp * CHUNK)
                nc.vector.tensor_copy(w3[:, cs, 0:1], u3[:, 0][:, cs, 0:1])
                nc.vector.tensor_tensor(
                    w3[:, cs, 1:], u3[:, 0][:, cs, 1:],
                    uf[:, cs, CHUNK:2 * CHUNK - 1], Alu.mult)
                nc.vector.tensor_tensor_scan(
                    t_sc[:, fs], data0=msk_sb[:, fs], data1=w[:, fs],
                    initial=1.0, op0=Alu.max, op1=Alu.mult)
            t3 = t_sc.rearrange("p (h c g) -> p h c g", h=2, g=CHUNK)
            nc.vector.tensor_tensor(t3[:, 1], t3[:, 0], u3[:, 1], Alu.mult)
            tta = ttpool.tile([128, SB, CHUNK], fp16, name="tta")
            ttb = ttpool.tile([128, SB, CHUNK], fp16, name="ttb")
            nc.sync.dma_start_transpose(tta[:], t_sc[:, :SB * CHUNK])
            nc.sync.dma_start_transpose(ttb[:], t_sc[:, SB * CHUNK:])
            r_ps = rps_pool.tile([128, 512], fp32, name="r_ps")
            for k2 in range(4):
                m = k2
                for b, ttx in enumerate((tta, ttb)):
                    nc.tensor.matmul(
                        r_ps[32 * m:32 * m + 32, :],
                        lhsT=gam_sb[:, b, :],
                        rhs=ttx[:, 4 * k2:4 * k2 + 4, :],
                        start=(b == 0), stop=(b == 1),
                        tile_position=(0, 32 * m))
            gi = sb                                # group of 16 chunks
            k = gi % 4
            if k == 0:
                stage_b.o_sb4 = opool.tile([128, 4 * 512], fp32, name="o_sb4")
            o_sb4 = stage_b.o_sb4
            # min(r + c0, 1) == 1 - relu((1 - c0) - r); emit y = relu(...) on
            # the Scalar engine (c0_sb holds 1 - c0) and let the host gather
            # finish with clip(1 - y, 0, 1) — moves this off the busy DVE
            nc.scalar.activation(
                o_sb4[:, k * 512:(k + 1) * 512], r_ps[:], Act.Relu,
                bias=c0_sb[:], scale=-1.0)
            if k == 3:
                G = gi // 4                        # supergroup of 64 chunks
                big = out_d.ap()[:, 8192 * G:8192 * (G + 1)]
                big = big.rearrange("c (k m f) -> c m k f", k=4, m=4)
                for m2 in range(4):
                    nc.sync.dma_start(
                        big[:, m2],
                        o_sb4[32 * m2:32 * m2 + 3, :])

        # one-stage software pipeline: stage_a(sb+1) is emitted before
        # stage_b(sb) so each engine's in-order queue always has ready work
        # (the DVE otherwise stalls between u-batches waiting on the scan's
        # cross-engine inputs)
        prev = None
        for sb in range(NSB):
            if (sb * SB) % QCH == 0:
                load_quarter((sb * SB) // QCH)
            u2 = stage_a(sb)
            if prev is not None:
                stage_b(prev[0], prev[1])
            prev = (sb, u2)
        stage_b(prev[0], prev[1])
    nc.compile()
    return nc


def _prep_host(coords, means, log_scales, rotations, raw_colors, raw_opacities):
    """Tiny host-side parameter preparation (float64 for coefficient accuracy)."""
    f64 = np.float64
    scales = np.exp(log_scales.astype(f64))
    sx2, sy2 = scales[:, 0] ** 2, scales[:, 1] ** 2
    cos_r = np.cos(rotations.astype(f64))
    sin_r = np.sin(rotations.astype(f64))
    a = cos_r ** 2 / (2 * sx2) + sin_r ** 2 / (2 * sy2)
    b = -sin_r * cos_r / (2 * sx2) + sin_r * cos_r / (2 * sy2)
    c = sin_r ** 2 / (2 * sx2) + cos_r ** 2 / (2 * sy2)
    opac = 1.0 / (1.0 + np.exp(-raw_opacities.astype(f64)))
    colors = 1.0 / (1.0 + np.exp(-raw_colors.astype(f64)))   # (N, 3)

    mx = means[:, 0].astype(f64) - 0.5
    my = means[:, 1].astype(f64) - 0.5
    # e = -(a dx^2 + 2 b dx dy + c dy^2) + ln(opac), expanded over centered
    # features [x^2, xy, y^2, x, y, 1]
    C6 = np.stack([
        -a,
        -2.0 * b,
        -c,
        2.0 * a * mx + 2.0 * b * my,
        2.0 * b * mx + 2.0 * c * my,
        -(a * mx ** 2 + 2.0 * b * mx * my + c * my ** 2) + np.log(opac),
    ]).astype(np.float32)                                    # (6, N)

    # device gaussian order per chunk: evens (0,2,..,254) then odds (1,3,..,255)
    # so that the pair-product scan input is a contiguous elementwise multiply
    perm = np.concatenate([np.arange(0, N, 2), np.arange(1, N, 2)])
    C6 = np.ascontiguousarray(C6[:, perm])

    gamp = np.zeros((N, 3), np.float64)                      # abel weights
    gamp[:N - 1] = colors[1:] - colors[:-1]
    gamp[N - 1] = 1.0 - colors[N - 1]
    gam = np.zeros((N, 32), np.float64)
    gam[:, :3] = gamp[perm]
    gam = gam.astype(np.float16)

    c0 = np.zeros((128, 1), np.float32)
    for m in range(4):
        c0[32 * m:32 * m + 3, 0] = (1.0 - colors[0]).astype(np.float32)

    msk = np.zeros((128, SCAN_F), np.float16)
    msk[:, ::CHUNK] = 1.0

    x = coords[:, :, 0].astype(f64) - 0.5                    # (H, W)
    y = coords[:, :, 1].astype(f64) - 0.5
    feats = np.stack([x * x, x * y, y * y, x, y, np.ones_like(x)])  # (6, H, W)
    feats = feats.reshape(6, NCORES, PIX).astype(np.float32)

    clip_needed = bool((opac > 0.99).any())
    return feats, C6, gam, c0, msk, clip_needed


def kernel(coords, means, log_scales, rotations, raw_colors, raw_opacities):
    from concourse.bass_utils import run_bass_kernel_spmd

    feats, C6, gam, c0, msk, clip_needed = _prep_host(
        coords, means, log_scales, rotations, raw_colors, raw_opacities)

    key = ("prog", clip_needed)
    if key not in _CACHE:
        _CACHE[key] = _build_program(clip_needed)
    nc = _CACHE[key]

    in_maps = [
        {"ft": np.ascontiguousarray(feats[:, k]), "c6": C6, "gam": gam,
         "c0": c0, "msk": msk}
        for k in range(NCORES)
    ]
    res = run_bass_kernel_spmd(nc, in_maps, list(range(NCORES)))
    out = np.stack([res.results[k]["out"] for k in range(NCORES)])  # (8, 3, PIX)
    out = out.reshape(NCORES, 3, ROWS_PER_CORE, W).transpose(0, 2, 3, 1)
    out = out.reshape(H, W, 3)
    # device emits y = relu((1 - c0) - rendered); min(r + c0, 1) = 1 - y
    return np.clip(1.0 - out, 0.0, 1.0).astype(np.float32)


# revision 39
# speedup vs baseline: 1.0318x; 1.0318x over previous
"""Trainium2 Bass kernel for the 2D Gaussian splatting model (nn_GaussianModel2D).

Math (per pixel p, gaussians n = 0..255 in order):
    e_n(p)   = -(a dx^2 + 2b dx dy + c dy^2) + ln(opac_n)      (quadratic in x,y)
    alpha_n  = exp(e_n)            (clip to 0.99 never binds for this input;
                                    checked on host, fallback applies it)
    u_n      = 1 - alpha_n
    scan_t   = prod_{k<=t} u_k = T_{t+1}  (inclusive cumprod)
    out_c    = clip(c0_c + sum_t gamma_{t,c} * scan_t, 0, 1)
    where gamma_{t,c} = col_{t+1,c} - col_{t,c} (t<255), gamma_{255,c} = 1 - col_{255,c}
    (Abel summation of sum_n w_n col_n + bg;  w_n = T_n - T_{n+1}, bg = T_256)

Device layout per core (1/8 of the image rows = 32768 pixels):
    - exponent: PE matmul, K=6 features [x'^2, x'y', y'^2, x', y', 1] (centered),
      fp32r, out (128 pix, 256 g) chunks in PSUM
    - alpha: ACT exp, PSUM->SBUF fp16, 4 chunks per call
    - u = 1 - alpha: DVE tensor_scalar fp16 (fast 2-byte mode)
    - scan_t: the DVE scan runs at ~2.1 ns/element, so the gaussian axis is
      permuted host-side to [evens | odds] per chunk and the cumprod is done
      over SHIFTED PAIR PRODUCTS, halving scan elements:
        w_0     = u_0,  w_k = u_{2k-1} * u_{2k}   (DVE tensor_tensor, 2B fast)
        S_{2k}  = cumprod(w)_k                    (segmented scan, see below)
        S_{2k+1}= S_{2k} * u_{2k+1}               (DVE tensor_tensor mult)
      (u_{2k-1} is read as a shifted in-tile view; the k=0 edge is patched by
      an 8-element copy per super-batch.  A divide-based variant would avoid
      the shift but AluOpType.divide is not a valid DVE TensorTensor ISA op.)
      The scan uses a segmented-reset formulation, ONE instruction per 8
      chunks: state = max(mask_t, state_prev) * w_t, where mask = 1.0 at each
      chunk's first slot and 0.0 elsewhere.  Since all intermediate products
      lie in (0, 1], max(1, state) = 1 restarts the cumprod exactly at
      128-column boundaries, so 8 independent per-pixel cumprods run in a
      single long instruction (per-instruction scan overhead dominated the
      old per-chunk version).
    - transpose scan (pix, g) -> (g, pix): ONE batched DMA xbar transpose per
      8 chunks ([128, 2048] -> [128, 16, 128] per-128-block transposes); the
      per-DMA issue cost on the SP sequencer (~1.2 us) dominated the old
      per-half version (512 issues -> 32).
    - rendered: PE matmul gamma^T @ scan^T accumulated in PSUM at partition
      offsets 32m (zero-padded gamma to 32 cols so PSUM is fully written),
      rhs strided over 4 chunks per instruction (512 moving columns)
    - final: DVE tensor_scalar (+c0, min 1) PSUM->SBUF staging [128, 2048],
      then one DMA per 32-partition block per 64 chunks (16 output DMAs)
"""

import numpy as np

H, W, N = 512, 512, 256
NCORES = 8
ROWS_PER_CORE = H // NCORES            # 64
PIX = ROWS_PER_CORE * W                # 32768 pixels per core
CHUNK = 128                            # pixels per matmul chunk
NCHUNK = PIX // CHUNK                  # 256
SB = 16                                # chunks per super-batch (scan+transpose)
NSB = NCHUNK // SB                     # 16
SCAN_F = SB * N // 2                   # 2048 (scan runs over pair products)

_CACHE = {}


def _build_program(apply_opacity_clip: bool):
    import concourse.bass as bass
    import concourse.bacc as bacc
    import concourse.tile as tile
    import concourse.mybir as mybir
    from contextlib import ExitStack

    fp32 = mybir.dt.float32
    fp32r = mybir.dt.float32r
    fp16 = mybir.dt.float16
    Alu = mybir.AluOpType
    Act = mybir.ActivationFunctionType

    nc = bacc.Bacc("TRN2", target_bir_lowering=False, debug=False,
                   num_devices=NCORES)

    ft_d = nc.dram_tensor("ft", [6, PIX], fp32r, kind="ExternalInput")
    c6_d = nc.dram_tensor("c6", [6, N], fp32r, kind="ExternalInput")
    gam_d = nc.dram_tensor("gam", [N, 32], fp16, kind="ExternalInput")
    c0_d = nc.dram_tensor("c0", [128, 1], fp32, kind="ExternalInput")
    msk_d = nc.dram_tensor("msk", [128, SCAN_F], fp16, kind="ExternalInput")
    out_d = nc.dram_tensor("out", [3, PIX], fp32, kind="ExternalOutput")

    with tile.TileContext(nc) as tc, ExitStack() as ctx:
        consts = ctx.enter_context(tc.tile_pool(name="consts", bufs=1))
        ftpool = ctx.enter_context(tc.tile_pool(name="ftp", bufs=3))
        apool = ctx.enter_context(tc.tile_pool(name="alpha", bufs=3))
        upool = ctx.enter_context(tc.tile_pool(name="u", bufs=2))
        vpool = ctx.enter_context(tc.tile_pool(name="v", bufs=3))
        tpool = ctx.enter_context(tc.tile_pool(name="tsc", bufs=3))
        ttpool = ctx.enter_context(tc.tile_pool(name="tt", bufs=2))
        opool = ctx.enter_context(tc.tile_pool(name="osb", bufs=2))
        eps_pool = ctx.enter_context(tc.tile_pool(name="eps", bufs=3, space="PSUM"))
        rps_pool = ctx.enter_context(tc.tile_pool(name="rps", bufs=2, space="PSUM"))

        c6_sb = consts.tile([6, N], fp32r)
        gam_sb = consts.tile([128, 2, 32], fp16)
        c0_sb = consts.tile([128, 1], fp32)
        msk_sb = consts.tile([128, SCAN_F], fp16)
        nc.sync.dma_start(c6_sb[:], c6_d[:])
        nc.sync.dma_start(gam_sb[:], gam_d.ap().rearrange("(b k) c -> k b c", k=128))
        nc.sync.dma_start(c0_sb[:], c0_d[:])
        nc.sync.dma_start(msk_sb[:], msk_d[:])

        QCH = NCHUNK // 4                          # chunks per ft quarter
        # Quarter 0 on the fast HWDGE queue (no transposes in flight yet, so
        # no ring conflict); quarters 1-3 issued upfront on the gpsimd SWDGE
        # queue, which keeps these large loads off the HWDGE ring the xbar
        # transposes use (concurrent big descriptors there corrupt in-flight
        # transposes).  The pool WAR semaphores naturally delay Q2/Q3 until
        # their buffer frees.
        ft_tiles = []

        def load_quarter(Q):
            # two half-loads with 8KB descriptors: the first super-batch's
            # matmuls unblock on the first half, and small descriptors don't
            # monopolize DMA engines against the in-flight xbar transposes
            ft_sb = ftpool.tile([6, QCH * CHUNK], fp32r, name="ftq")
            src = ft_d.ap()[:, Q * QCH * CHUNK:(Q + 1) * QCH * CHUNK]
            half = QCH * CHUNK // 2
            nc.gpsimd.dma_start(ft_sb[:, :half], src[:, :half],
                                max_dma_last_dim=2048)
            nc.gpsimd.dma_start(ft_sb[:, half:], src[:, half:],
                                max_dma_last_dim=2048)
            ft_tiles.append(ft_sb)

        def stage_a(sb):
            ft_sb = ft_tiles[(sb * SB) // QCH]
            u2 = upool.tile([128, SB * N], fp16, name="u2")
            for h in range(SB // 4):
                e_ps = eps_pool.tile([128, 4 * N], fp32, name="e_ps")
                for q in range(4):
                    j = (sb * SB + h * 4 + q) % QCH
                    nc.tensor.matmul(
                        e_ps[:, q * N:(q + 1) * N],
                        lhsT=ft_sb[:, j * CHUNK:(j + 1) * CHUNK],
                        rhs=c6_sb[:],
                        start=True, stop=True)
                al = apool.tile([128, 4 * N], fp16, name="al")
                nc.scalar.activation(al[:], e_ps[:], Act.Exp)
                if apply_opacity_clip:
                    nc.vector.tensor_scalar(al[:], al[:], 0.99, None, Alu.min)
                nc.vector.tensor_scalar(
                    u2[:, h * 4 * N:(h + 1) * 4 * N], al[:], -1.0, 1.0,
                    Alu.mult, Alu.add)
            return u2

        def stage_b(sb, u2):
            # per-chunk gaussian layout (after host permutation of C6 cols):
            # u2[:, c*256 : c*256+128] = u_even, u2[:, c*256+128 : +256] = u_odd
            u3 = u2.rearrange("p (c b g) -> p b c g", b=2, g=CHUNK)
            uf = u2.rearrange("p (c g) -> p c g", g=2 * CHUNK)
            w = vpool.tile([128, SCAN_F], fp16, name="w")
            w3 = w.rearrange("p (c g) -> p c g", g=CHUNK)
            nc.vector.tensor_copy(w3[:, :, 0:1], u3[:, 0][:, :, 0:1])
            nc.vector.tensor_tensor(
                w3[:, :, 1:], u3[:, 0][:, :, 1:], uf[:, :, CHUNK:2 * CHUNK - 1],
                Alu.mult)
            t_sc = tpool.tile([128, 2 * SCAN_F], fp16, name="t_sc")
            nc.vector.tensor_tensor_scan(
                t_sc[:, :SCAN_F], data0=msk_sb[:], data1=w[:], initial=1.0,
                op0=Alu.max, op1=Alu.mult)
            t3 = t_sc.rearrange("p (h c g) -> p h c g", h=2, g=CHUNK)
            nc.vector.tensor_tensor(t3[:, 1], t3[:, 0], u3[:, 1], Alu.mult)
            tta = ttpool.tile([128, SB, CHUNK], fp16, name="tta")
            ttb = ttpool.tile([128, SB, CHUNK], fp16, name="ttb")
            nc.sync.dma_start_transpose(tta[:], t_sc[:, :SB * CHUNK])
            nc.sync.dma_start_transpose(ttb[:], t_sc[:, SB * CHUNK:])
            r_ps = rps_pool.tile([128, 512], fp32, name="r_ps")
            for k2 in range(4):
                m = k2
                for b, ttx in enumerate((tta, ttb)):
                    nc.tensor.matmul(
                        r_ps[32 * m:32 * m + 32, :],
                        lhsT=gam_sb[:, b, :],
                        rhs=ttx[:, 4 * k2:4 * k2 + 4, :],
                        start=(b == 0), stop=(b == 1),
                        tile_position=(0, 32 * m))
            gi = sb                                # group of 16 chunks
            k = gi % 4
            if k == 0:
                stage_b.o_sb4 = opool.tile([128, 4 * 512], fp32, name="o_sb4")
            o_sb4 = stage_b.o_sb4
            # min(r + c0, 1) == 1 - relu((1 - c0) - r); emit y = relu(...) on
            # the Scalar engine (c0_sb holds 1 - c0) and let the host gather
            # finish with clip(1 - y, 0, 1) — moves this off the busy DVE
            nc.scalar.activation(
                o_sb4[:, k * 512:(k + 1) * 512], r_ps[:], Act.Relu,
                bias=c0_sb[:], scale=-1.0)
            if k == 3:
                G = gi // 4                        # supergroup of 64 chunks
                big = out_d.ap()[:, 8192 * G:8192 * (G + 1)]
                big = big.rearrange("c (k m f) -> c m k f", k=4, m=4)
                for m2 in range(4):
                    nc.sync.dma_start(
                        big[:, m2],
                        o_sb4[32 * m2:32 * m2 + 3, :])

        # one-stage software pipeline: stage_a(sb+1) is emitted before
        # stage_b(sb) so each engine's in-order queue always has ready work
        # (the DVE otherwise stalls between u-batches waiting on the scan's
        # cross-engine inputs)
        prev = None
        for sb in range(NSB):
            if (sb * SB) % QCH == 0:
                load_quarter((sb * SB) // QCH)
            u2 = stage_a(sb)
            if prev is not None:
                stage_b(prev[0], prev[1])
            prev = (sb, u2)
        stage_b(prev[0], prev[1])
    nc.compile()
    return nc


def _prep_host(coords, means, log_scales, rotations, raw_colors, raw_opacities):
    """Tiny host-side parameter preparation (float64 for coefficient accuracy)."""
    f64 = np.float64
    scales = np.exp(log_scales.astype(f64))
    sx2, sy2 = scales[:, 0] ** 2, scales[:, 1] ** 2
    cos_r = np.cos(rotations.astype(f64))
    sin_r = np.sin(rotations.astype(f64))
    a = cos_r ** 2 / (2 * sx2) + sin_r ** 2 / (2 * sy2)
    b = -sin_r * cos_r / (2 * sx2) + sin_r * cos_r / (2 * sy2)
    c = sin_r ** 2 / (2 * sx2) + cos_r ** 2 / (2 * sy2)
    opac = 1.0 / (1.0 + np.exp(-raw_opacities.astype(f64)))
    colors = 1.0 / (1.0 + np.exp(-raw_colors.astype(f64)))   # (N, 3)

    mx = means[:, 0].astype(f64) - 0.5
    my = means[:, 1].astype(f64) - 0.5
    # e = -(a dx^2 + 2 b dx dy + c dy^2) + ln(opac), expanded over centered
    # features [x^2, xy, y^2, x, y, 1]
    C6 = np.stack([
        -a,
        -2.0 * b,
        -c,
        2.0 * a * mx + 2.0 * b * my,
        2.0 * b * mx + 2.0 * c * my,
        -(a * mx ** 2 + 2.0 * b * mx * my + c * my ** 2) + np.log(opac),
    ]).astype(np.float32)                                    # (6, N)

    # device gaussian order per chunk: evens (0,2,..,254) then odds (1,3,..,255)
    # so that the pair-product scan input is a contiguous elementwise multiply
    perm = np.concatenate([np.arange(0, N, 2), np.arange(1, N, 2)])
    C6 = np.ascontiguousarray(C6[:, perm])

    gamp = np.zeros((N, 3), np.float64)                      # abel weights
    gamp[:N - 1] = colors[1:] - colors[:-1]
    gamp[N - 1] = 1.0 - colors[N - 1]
    gam = np.zeros((N, 32), np.float64)
    gam[:, :3] = gamp[perm]
    gam = gam.astype(np.float16)

    c0 = np.zeros((128, 1), np.float32)
    for m in range(4):
        c0[32 * m:32 * m + 3, 0] = (1.0 - colors[0]).astype(np.float32)

    msk = np.zeros((128, SCAN_F), np.float16)
    msk[:, ::CHUNK] = 1.0

    x = coords[:, :, 0].astype(f64) - 0.5                    # (H, W)
    y = coords[:, :, 1].astype(f64) - 0.5
    feats = np.stack([x * x, x * y, y * y, x, y, np.ones_like(x)])  # (6, H, W)
    feats = feats.reshape(6, NCORES, PIX).astype(np.float32)

    clip_needed = bool((opac > 0.99).any())
    return feats, C6, gam, c0, msk, clip_needed


def kernel(coords, means, log_scales, rotations, raw_colors, raw_opacities):
    from concourse.bass_utils import run_bass_kernel_spmd

    feats, C6, gam, c0, msk, clip_needed = _prep_host(
        coords, means, log_scales, rotations, raw_colors, raw_opacities)

    key = ("prog", clip_needed)
    if key not in _CACHE:
        _CACHE[key] = _build_program(clip_needed)
    nc = _CACHE[key]

    in_maps = [
        {"ft": np.ascontiguousarray(feats[:, k]), "c6": C6, "gam": gam,
         "c0": c0, "msk": msk}
        for k in range(NCORES)
    ]
    res = run_bass_kernel_spmd(nc, in_maps, list(range(NCORES)))
    out = np.stack([res.results[k]["out"] for k in range(NCORES)])  # (8, 3, PIX)
    out = out.reshape(NCORES, 3, ROWS_PER_CORE, W).transpose(0, 2, 3, 1)
    out = out.reshape(H, W, 3)
    # device emits y = relu((1 - c0) - rendered); min(r + c0, 1) = 1 - y
    return np.clip(1.0 - out, 0.0, 1.0).astype(np.float32)
